# revision 6
# baseline (speedup 1.0000x reference)
"""Contrastive loss (SimCLR-style NT-Xent) Trainium2 kernel.

Full inputs z1, z2: [4096, 1024] f32. Output: scalar f32 loss.

Strategy (8 NeuronCores, SPMD, no collectives), exploiting sim symmetry:
  - Host: L2-normalize rows of reps = concat(z1, z2)  [8192, 1024] (f32),
    transpose to repsT [1024, 8192], scale by 256 and cast to fp8e4m3.
  - sim = reps @ reps.T is symmetric, so each ordered pair only needs to be
    computed once. Circulant block scheme: core c computes rows
    R_c = [c*1024, (c+1)*1024) against column blocks d = 0..4, i.e. actual
    columns [c*1024, c*1024 + 5120) mod 8192 — 5/8 of the full GEMM.
      * d=0 (cols R_c): internal pairs; row-sums cover both orders. No
        col-sums needed.
      * d=1..3: the mirror blocks [R_{c+d}, R_c] are computed by nobody, so
        core c also exports per-column sums of exp over these blocks
        (partition reduction via a ones-matmul on the PE, accumulated in
        PSUM across the 8 row tiles); the host adds them into S_j for the
        rows owned by cores c+1..c+3.
      * d=4: cores c and c+4 both compute their mutual block, so row-sums
        alone cover both sides. The positive-pair diagonal lives here
        (rot col 4096 + r for local row r); the self diagonal in d=0.
  - Per core the program is identical (SPMD); feeding core c its columns
    rotated by p0 = c*1024 puts the self diagonal at rot cols [0, 1024)
    and the positive diagonal at rot cols [4096, 5120) for every core.
  - Device, per (m-tile of 128 rows, n-chunk of 1024 cols): fp8 DoubleRow
    matmuls accumulate K=1024 in 4 instructions per 512-col PSUM bank; ACT
    exp(s*x - 10) with fused per-row accumulation (accum_out). On chunks
    1..3 a bf16 ones-matmul (stationary all-ones [128,128]) reduces the
    exp tile along partitions into a per-chunk PSUM accumulator (every
    output partition holds the same col-sums; row 0 is DMAed out). The
    ones-matmul for tile t is emitted after the matmul group of tile t+1
    so the in-order PE queue never stalls waiting for ACT.
  - Host: S_i = (local row sums) + (col-sum contributions from 3 cores);
    T = S - e_self + e_pos; loss = mean(ln(T) - (pos - 10)).
"""

import time
from contextlib import ExitStack

import numpy as np
import ml_dtypes

import concourse.bass as bass
import concourse.tile as tile
from concourse import bacc
from concourse import mybir
from concourse import bass_utils
from concourse.masks import make_identity

B = 4096
D = 1024
S = 2 * B  # 8192 rows/cols of sim
NCORES = 8
ROWS_PER_CORE = S // NCORES  # 1024
P = 128
M_TILES = ROWS_PER_CORE // P  # 8
K_TILES = D // P  # 8
N_CHUNK = 1024  # two PSUM banks per (m, chunk) tile
N_CHUNKS = 5  # symmetric scheme: blocks d=0..4 only
N_COLS = N_CHUNKS * N_CHUNK  # 5120 columns per core
N_HALF = 512  # max matmul moving free dim into one PSUM bank
CS_CHUNKS = (1, 2, 3)  # chunks whose col-sums are exported
INV_T = 10.0  # 1 / temperature
EPS = 1e-12
FP8_SCALE = 256.0  # input scale: keeps fp8e4m3 operands in their sweet spot
SIM_SCALE = INV_T / (FP8_SCALE * FP8_SCALE)  # exp(SIM_SCALE * raw - INV_T)

_FP32 = mybir.dt.float32
_FP8 = mybir.dt.float8e4
_BF16 = mybir.dt.bfloat16
_FP8_NP = mybir.dt.np(_FP8)


def _build_bass():
    # Bacc (not raw Bass): its compile() runs generate_event_semaphores,
    # which splits multi-semaphore waits into standalone EventSemaphore
    # instructions — engine instructions can encode only one wait.
    nc = bacc.Bacc("TRN2", debug=False, num_devices=NCORES, enable_partition_id=False)
    # lhsT blocked per m-tile on the host: [m, kt, p, col] so each m-block is
    # one contiguous 128KB DMA and the PE can ramp as soon as block 0 lands.
    lhsT = nc.dram_tensor(
        "lhst", [M_TILES, K_TILES, P, P], _FP8, kind="ExternalInput"
    ).ap()
    # brot blocked per 512-column half on the host: [half, p, kt, col] so
    # each partition reads 4KB contiguous runs per half-DMA.
    brot = nc.dram_tensor(
        "brot", [N_COLS // N_HALF, P, K_TILES, N_HALF], _FP8, kind="ExternalInput"
    ).ap()
    # Raw reductions out; the tiny final combine (a few K flops) runs on the
    # host, which avoids a 1.3us ACT table switch (Ln) in the device tail.
    sums_out = nc.dram_tensor(
        "sums", [P, M_TILES * N_CHUNKS], _FP32, kind="ExternalOutput"
    ).ap()
    diag_out = nc.dram_tensor(
        "diag", [P, 2 * M_TILES], _FP32, kind="ExternalOutput"
    ).ap()
    # Per-column sums of exp over chunks 1..3 (rot cols [1024, 4096)).
    cs_out = nc.dram_tensor(
        "colsums", [1, len(CS_CHUNKS) * N_CHUNK], _FP32, kind="ExternalOutput"
    ).ap()

    # Pre-TileContext const region (same pattern as Bass.__init__'s
    # const_aps): values read by hot-loop instructions with no tracked
    # dependency, so they add no per-instruction sync waits. Instead of a
    # full all-engine barrier (~3us butterfly), hand off with one semaphore
    # to the only consumers (ACT reads the bias const, DVE the identity,
    # PE the ones block).
    bias_th = nc.alloc_sbuf_tensor("const-f32-neg10", [P, 1], _FP32)
    nc.gpsimd.memset(bias_th.ap(), -INV_T)
    nc.const_aps.aps[(_FP32, -INV_T)] = bias_th.ap()
    ident_th = nc.alloc_sbuf_tensor("identity-f32", [P, P], _FP32)
    nc.gpsimd.memset(ident_th.ap(), 0.0)
    ones_th = nc.alloc_sbuf_tensor("ones-bf16", [P, P], _BF16)
    nc.gpsimd.memset(ones_th.ap(), 1.0)
    ident_inst = nc.gpsimd.affine_select(
        out=ident_th.ap(),
        in_=ident_th.ap(),
        compare_op=mybir.AluOpType.not_equal,
        fill=1.0,
        base=0,
        pattern=[[-1, P]],
        channel_multiplier=1,
    )
    const_sem = nc.alloc_semaphore("const-ready")
    ident_inst.then_inc(const_sem, 1)
    nc.vector.wait_ge(const_sem, 1)
    nc.scalar.wait_ge(const_sem, 1)
    nc.tensor.wait_ge(const_sem, 1)

    with tile.TileContext(nc) as tc:
        _body(tc, lhsT, brot, sums_out, diag_out, cs_out, ident_th.ap(), ones_th.ap())
    nc.compile()
    return nc


def _body(tc, lhsT, brot, sums_out, diag_out, cs_out, ident, ones):
    nc = tc.nc
    AF = mybir.ActivationFunctionType

    # DRAM views with partition dim first: [p, kt, ...]
    a_view = lhsT.rearrange("m k p c -> p m k c")  # [128, 8, 8, 128]

    ctx = ExitStack()
    singles = ctx.enter_context(tc.tile_pool(name="singles", bufs=1))
    bpool = ctx.enter_context(tc.tile_pool(name="bchunks", bufs=3))
    # 3 tiles x 2 banks for the matmul pipeline + 2 banks for the col-sum
    # accumulator = all 8 PSUM banks.
    pspool = ctx.enter_context(tc.tile_pool(name="psum", bufs=3, space="PSUM"))
    cspool = ctx.enter_context(tc.tile_pool(name="psum-cs", bufs=1, space="PSUM"))
    # Exp elementwise outputs feed the ones-matmul col reduction (chunks
    # 1..3) and are garbage elsewhere; bf16 halves their SBUF footprint and
    # is plenty for a 1024-term f32 PSUM-accumulated sum.
    epool = ctx.enter_context(tc.tile_pool(name="exps", bufs=8))
    # Single-use slots for the 16 diagonal extractions.
    scratch = ctx.enter_context(tc.tile_pool(name="scratch", bufs=16))

    # Resident stationary operand: all local rows, transposed. SBUF layout
    # [p, kt, m*128+col]; m-block 0 is loaded before the first b chunk, the
    # rest right after it (the PE consumes m-blocks at ~2us each, so they
    # arrive well ahead).
    a_t = singles.tile([P, K_TILES, ROWS_PER_CORE], _FP8)

    def load_a_block(m):
        nc.sync.dma_start(
            out=a_t[:, :, m * P : (m + 1) * P], in_=a_view[:, m, :, :]
        )

    load_a_block(0)

    # Per-row partial sums: column m*N_CHUNKS + nch. Disjoint-column writes
    # carry no WAW dependencies between the exps.
    sums = singles.tile([P, M_TILES * N_CHUNKS], _FP32)
    # Raw (pre-exp, scaled) diagonal values: cols [0:8] positive, [8:16] self.
    diag = singles.tile([P, 2 * M_TILES], _FP32)
    # SBUF staging for the col-sums (DMA cannot read PSUM directly).
    cs_sb = singles.tile([1, len(CS_CHUNKS) * N_CHUNK], _FP32)

    # Deferred ones-matmul emissions: the col reduction of tile t is queued
    # on the PE after tile t+1's matmul group so the PE never waits for
    # ACT(t); flushing also emits the cs DMA when a chunk completes.
    pending = []

    def flush_pending():
        while pending:
            pending.pop(0)()

    for nch in range(N_CHUNKS):
        b_t = bpool.tile([P, K_TILES, N_CHUNK], _FP8)
        # Two half-loads (columns) so matmuls on the first PSUM bank can
        # start while the second half is still arriving.
        nc.sync.dma_start(out=b_t[:, :, 0:N_HALF], in_=brot[2 * nch])
        nc.sync.dma_start(out=b_t[:, :, N_HALF:N_CHUNK], in_=brot[2 * nch + 1])
        if nch == 0:
            for mb in range(1, M_TILES):
                load_a_block(mb)
        for m in range(M_TILES):
            ps = pspool.tile([P, N_CHUNK], _FP32)
            col = m * N_CHUNKS + nch
            for half in range(N_CHUNK // N_HALF):
                hs = slice(half * N_HALF, (half + 1) * N_HALF)
                for kt in range(0, K_TILES, 2):
                    nc.tensor.matmul(
                        ps[:, hs],
                        a_t[:, kt : kt + 2, m * P : (m + 1) * P],
                        b_t[:, kt : kt + 2, hs],
                        start=(kt == 0),
                        stop=(kt == K_TILES - 2),
                        perf_mode=mybir.MatmulPerfMode.DoubleRow,
                    )
            flush_pending()
            # exp over both PSUM banks at once; fused per-row accumulation.
            # All PE-group RAW waits share one semaphore.
            e_t = epool.tile([P, N_CHUNK], _BF16)
            nc.scalar.activation(
                out=e_t,
                in_=ps,
                func=AF.Exp,
                bias=-INV_T,
                scale=SIM_SCALE,
                accum_out=sums[:, col : col + 1],
            )
            if nch in CS_CHUNKS:
                # Col-sum reduction: ones[128,128].T @ e_t accumulates the
                # partition sums into every output partition; PSUM carries
                # the accumulation across the chunk's 8 m-tiles.
                if m == 0:
                    cs_t = cspool.tile([P, N_CHUNK], _FP32)
                else:
                    cs_t = pending_cs

                def emit(cs_t=cs_t, e_t=e_t, m=m, nch=nch):
                    # One matmul per 512-col half: a matmul output must not
                    # cross a PSUM bank boundary.
                    for half in range(N_CHUNK // N_HALF):
                        hs = slice(half * N_HALF, (half + 1) * N_HALF)
                        nc.tensor.matmul(
                            cs_t[:, hs],
                            ones,
                            e_t[:, hs],
                            start=(m == 0),
                            stop=(m == M_TILES - 1),
                        )
                    if m == M_TILES - 1:
                        # Park row 0 in SBUF on the (mostly idle) DVE so the
                        # PSUM accumulator can recycle for the next chunk.
                        ci = CS_CHUNKS.index(nch)
                        nc.vector.tensor_scalar_mul(
                            cs_sb[:, ci * N_CHUNK : (ci + 1) * N_CHUNK],
                            cs_t[0:1, :],
                            1.0,
                        )

                pending.append(emit)
                pending_cs = cs_t
            # Diagonal extraction on the two special chunks. In rotated
            # coords, m-tile m's self diagonal lives at rot cols
            # [m*128, (m+1)*128) -> chunk 0; the positive-pair diagonal at
            # rot cols [4096 + m*128, ...) -> chunk 4.
            dcol = None
            if nch == 0:
                dcol = M_TILES + m
            elif nch == 4:
                dcol = m
            if dcol is not None:
                # DVE extracts the raw f32 diagonal straight from PSUM
                # (identity mul + reduce); Bacc's generate_event_semaphores
                # legalizes the resulting extra WAR wait on the recycling
                # matmul, and this keeps the ACT engine (the pipeline's
                # second-busiest) free of copy work.
                off = m * P
                diag_t = scratch.tile([P, P], _FP32)
                nc.vector.tensor_mul(diag_t, ps[:, off : off + P], ident)
                nc.vector.reduce_sum(
                    diag[:, dcol : dcol + 1], diag_t, axis=mybir.AxisListType.X
                )

    flush_pending()
    nc.sync.dma_start(out=diag_out, in_=diag)
    nc.sync.dma_start(out=sums_out, in_=sums)
    nc.sync.dma_start(out=cs_out, in_=cs_sb)
    ctx.close()


_NC_CACHE = {}


def _get_nc():
    if "nc" not in _NC_CACHE:
        _NC_CACHE["nc"] = _build_bass()
    return _NC_CACHE["nc"]


def _make_in_maps(z1, z2):
    z1 = np.asarray(z1, dtype=np.float32)
    z2 = np.asarray(z2, dtype=np.float32)
    z = np.concatenate([z1, z2], axis=0)  # [8192, 1024]
    nrm = np.sqrt(np.sum(z * z, axis=1, keepdims=True, dtype=np.float32))
    n = z / np.maximum(nrm, EPS)
    repsT = np.ascontiguousarray(n.T * FP8_SCALE).astype(_FP8_NP)  # [1024, 8192]
    in_maps = []
    for c in range(NCORES):
        p0 = c * ROWS_PER_CORE
        rolled = np.concatenate([repsT[:, p0:], repsT[:, :p0]], axis=1)
        cols = rolled[:, :N_COLS]  # rot cols [0, 5120)
        lhsT_c = repsT[:, c * ROWS_PER_CORE : (c + 1) * ROWS_PER_CORE]
        # Block per m-tile: [m, kt, p, col]
        lhsT_blk = np.ascontiguousarray(
            lhsT_c.reshape(K_TILES, P, M_TILES, P).transpose(2, 0, 1, 3)
        )
        # Block per 512-col half: [half, p, kt, col]
        b_blk = np.ascontiguousarray(
            cols.reshape(K_TILES, P, N_COLS // N_HALF, N_HALF).transpose(2, 1, 0, 3)
        )
        in_maps.append({"lhst": lhsT_blk, "brot": b_blk})
    return in_maps


def _combine(results):
    # Per row i: S_i = local row sums + col-sum contributions from the three
    # cores whose d=1..3 blocks hit column i. Then
    #   T_i = S_i - e_self_i + e_pos_i;  loss_row = ln(T_i) - (pos_i - 10)
    # with pos_i - 10 = SIM_SCALE*draw - 10. A few K flops; done in f64.
    S_loc = np.empty(S, dtype=np.float64)
    CS = np.zeros(S, dtype=np.float64)
    draw = np.empty(S, dtype=np.float64)
    dself = np.empty(S, dtype=np.float64)
    ncs = len(CS_CHUNKS) * N_CHUNK
    for c, r in enumerate(results):
        stot = r["sums"].astype(np.float64).reshape(P, M_TILES, N_CHUNKS).sum(axis=2)
        sl = slice(c * ROWS_PER_CORE, (c + 1) * ROWS_PER_CORE)
        S_loc[sl] = stot.T.reshape(-1)  # local row l = m*128 + p
        diag = r["diag"].astype(np.float64)
        draw[sl] = diag[:, :M_TILES].T.reshape(-1)
        dself[sl] = diag[:, M_TILES:].T.reshape(-1)
        cs = r["colsums"].astype(np.float64).reshape(-1)  # rot cols [1024, 4096)
        idx = (np.arange(ncs) + c * ROWS_PER_CORE + N_CHUNK) % S
        CS[idx] += cs
    Stot = S_loc + CS
    e_pos = np.exp(SIM_SCALE * draw - INV_T)
    e_self = np.exp(SIM_SCALE * dself - INV_T)
    loss_rows = np.log(Stot - e_self + e_pos) - (SIM_SCALE * draw - INV_T)
    return np.array(loss_rows.mean(), dtype=np.float32)


def run_traced(z1, z2, **spmd_kwargs):
    """Run on HW with profiling; returns (loss, BassKernelResults)."""
    nc = _get_nc()
    in_maps = _make_in_maps(z1, z2)
    res = bass_utils.run_bass_kernel_spmd(
        nc, in_maps, core_ids=list(range(NCORES)), trace=True, **spmd_kwargs
    )
    return _combine(res.results), res


def kernel(z1, z2):
    nc = _get_nc()
    in_maps = _make_in_maps(z1, z2)
    last_err = None
    for _attempt in range(3):
        try:
            res = bass_utils.run_bass_kernel_spmd(
                nc, in_maps, core_ids=list(range(NCORES))
            )
            return _combine(res.results)
        except Exception as e:  # transient device wedge: retry
            last_err = e
            time.sleep(2.0)
    raise last_err


# revision 7
# speedup vs baseline: 1.0566x; 1.0566x over previous
"""Contrastive loss (SimCLR-style NT-Xent) Trainium2 kernel.

Full inputs z1, z2: [4096, 1024] f32. Output: scalar f32 loss.

Strategy (8 NeuronCores, SPMD, no collectives), exploiting sim symmetry:
  - Host: L2-normalize rows of reps = concat(z1, z2)  [8192, 1024] (f32),
    transpose to repsT [1024, 8192], scale by 256 and cast to fp8e4m3.
  - sim = reps @ reps.T is symmetric, so each unordered pair only needs to
    be computed once. Circulant block scheme: core c computes rows
    R_c = [c*1024, (c+1)*1024) against column blocks d = 0..4, i.e. actual
    columns [c*1024, c*1024 + 5120) mod 8192 — 5/8 of the full GEMM.
      * d=0 (cols R_c): internal pairs; row-sums cover both orders.
      * d=1..3: the mirror blocks [R_{c+d}, R_c] are computed by nobody, so
        core c also exports per-column sums of exp over these blocks; the
        host adds them into S_j for the rows owned by cores c+1..c+3.
      * d=4: cores c and c+4 both compute their mutual block, so row-sums
        alone cover both sides. The positive-pair diagonal lives here
        (rot col 4096 + r for local row r); the self diagonal in d=0.
  - Per core the program is identical (SPMD); feeding core c its columns
    rotated by p0 = c*1024 puts the self diagonal at rot cols [0, 1024)
    and the positive diagonal at rot cols [4096, 5120) for every core.
  - Device, per (m-tile of 128 rows, n-chunk of 1024 cols): fp8 DoubleRow
    matmuls accumulate K=1024 in 4 instructions per 512-col PSUM bank; ACT
    exp with fused per-row accumulation (accum_out). Chunks 0/4 use
    exp(s*x - 10) with a garbage elementwise output; chunks 1..3 use
    exp(s*x - 1) written as fp8e4m3 (values ~e^{10 cos} stay in fp8 range;
    the host rescales by e^-9) into per-PAIR tiles [128, 2, 1024] so that a
    single fp8 DoubleRow ones-matmul per pair reduces TWO m-tiles along
    partitions at once into the per-chunk PSUM col-sum accumulator. Each
    ones-matmul is emitted after the NEXT tile's matmul group so the
    in-order PE queue never stalls waiting for ACT.
  - Chunk processing order [0, 4, 1, 2, 3]: both diagonals are finished and
    shipped 2/5 into the kernel, keeping the tail short.
  - DMA layouts are fully contiguous per partition (1KB a-blocks, 4KB
    b-halves) for large-packet transfers.
  - Host: S_i = (local row sums) + (col-sum contributions from 3 cores);
    T = S - e_self + e_pos; loss = mean(ln(T) - (pos - 10)).
"""

import time
from contextlib import ExitStack

import numpy as np
import ml_dtypes

import concourse.bass as bass
import concourse.tile as tile
from concourse import bacc
from concourse import mybir
from concourse import bass_utils
from concourse.masks import make_identity

B = 4096
D = 1024
S = 2 * B  # 8192 rows/cols of sim
NCORES = 8
ROWS_PER_CORE = S // NCORES  # 1024
P = 128
M_TILES = ROWS_PER_CORE // P  # 8
K_TILES = D // P  # 8
N_CHUNK = 1024  # two PSUM banks per (m, chunk) tile
N_CHUNKS = 5  # symmetric scheme: blocks d=0..4 only
N_COLS = N_CHUNKS * N_CHUNK  # 5120 columns per core
N_HALF = 512  # max matmul moving free dim into one PSUM bank
CS_CHUNKS = (1, 2, 3)  # chunks whose col-sums are exported
CHUNK_ORDER = (0, 4, 1, 2, 3)  # diagonals first -> short tail
INV_T = 10.0  # 1 / temperature
CS_BIAS = 1.0  # cs chunks compute exp(s*x - CS_BIAS) to fit fp8e4m3
EPS = 1e-12
FP8_SCALE = 256.0  # input scale: keeps fp8e4m3 operands in their sweet spot
SIM_SCALE = INV_T / (FP8_SCALE * FP8_SCALE)  # exp(SIM_SCALE * raw - bias)

_FP32 = mybir.dt.float32
_FP8 = mybir.dt.float8e4
_BF16 = mybir.dt.bfloat16
_FP8_NP = mybir.dt.np(_FP8)


def _build_bass():
    # Bacc (not raw Bass): its compile() runs generate_event_semaphores,
    # which splits multi-semaphore waits into standalone EventSemaphore
    # instructions — engine instructions can encode only one wait.
    nc = bacc.Bacc("TRN2", debug=False, num_devices=NCORES, enable_partition_id=False)
    # lhsT stored [p, m, kt, col]: both the DMA src and dst are contiguous
    # 1KB runs per partition per m-block.
    lhsT = nc.dram_tensor(
        "lhst", [P, M_TILES, K_TILES, P], _FP8, kind="ExternalInput"
    ).ap()
    # brot blocked per 512-column half: [half, p, kt, col]; contiguous 4KB
    # per partition per half.
    brot = nc.dram_tensor(
        "brot", [N_COLS // N_HALF, P, K_TILES, N_HALF], _FP8, kind="ExternalInput"
    ).ap()
    # Raw reductions out; the tiny final combine (a few K flops) runs on the
    # host, which avoids a 1.3us ACT table switch (Ln) in the device tail.
    sums_out = nc.dram_tensor(
        "sums", [P, M_TILES * N_CHUNKS], _FP32, kind="ExternalOutput"
    ).ap()
    diag_out = nc.dram_tensor(
        "diag", [P, 2 * M_TILES], _FP32, kind="ExternalOutput"
    ).ap()
    # Per-column sums of exp over chunks 1..3 (rot cols [1024, 4096)).
    cs_out = nc.dram_tensor(
        "colsums", [1, len(CS_CHUNKS) * N_CHUNK], _FP32, kind="ExternalOutput"
    ).ap()

    # Pre-TileContext const region (same pattern as Bass.__init__'s
    # const_aps): values read by hot-loop instructions with no tracked
    # dependency, so they add no per-instruction sync waits. Instead of a
    # full all-engine barrier (~3us butterfly), hand off with one semaphore
    # to the only consumers (ACT reads the bias consts, DVE the identity,
    # PE the ones block).
    bias_th = nc.alloc_sbuf_tensor("const-f32-neg10", [P, 1], _FP32)
    nc.gpsimd.memset(bias_th.ap(), -INV_T)
    nc.const_aps.aps[(_FP32, -INV_T)] = bias_th.ap()
    bias2_th = nc.alloc_sbuf_tensor("const-f32-csbias", [P, 1], _FP32)
    nc.gpsimd.memset(bias2_th.ap(), -CS_BIAS)
    nc.const_aps.aps[(_FP32, -CS_BIAS)] = bias2_th.ap()
    ident_th = nc.alloc_sbuf_tensor("identity-f32", [P, P], _FP32)
    nc.gpsimd.memset(ident_th.ap(), 0.0)
    ones_th = nc.alloc_sbuf_tensor("ones-fp8", [P, 2, P], _FP8)
    nc.gpsimd.memset(ones_th.ap(), 1.0)
    ident_inst = nc.gpsimd.affine_select(
        out=ident_th.ap(),
        in_=ident_th.ap(),
        compare_op=mybir.AluOpType.not_equal,
        fill=1.0,
        base=0,
        pattern=[[-1, P]],
        channel_multiplier=1,
    )
    const_sem = nc.alloc_semaphore("const-ready")
    ident_inst.then_inc(const_sem, 1)
    nc.vector.wait_ge(const_sem, 1)
    nc.scalar.wait_ge(const_sem, 1)
    nc.tensor.wait_ge(const_sem, 1)

    with tile.TileContext(nc) as tc:
        _body(tc, lhsT, brot, sums_out, diag_out, cs_out, ident_th.ap(), ones_th.ap())
    nc.compile()
    return nc


def _body(tc, lhsT, brot, sums_out, diag_out, cs_out, ident, ones):
    nc = tc.nc
    AF = mybir.ActivationFunctionType

    ctx = ExitStack()
    singles = ctx.enter_context(tc.tile_pool(name="singles", bufs=1))
    bpool = ctx.enter_context(tc.tile_pool(name="bchunks", bufs=3))
    # 3 tiles x 2 banks for the matmul pipeline + 2 banks for the col-sum
    # accumulator = all 8 PSUM banks.
    pspool = ctx.enter_context(tc.tile_pool(name="psum", bufs=3, space="PSUM"))
    cspool = ctx.enter_context(tc.tile_pool(name="psum-cs", bufs=1, space="PSUM"))
    # Exp outputs per m-tile PAIR: [p, 2, 1024] fp8 so one DoubleRow
    # ones-matmul reduces both tiles along partitions at once.
    epool = ctx.enter_context(tc.tile_pool(name="exps", bufs=4))
    # Single-use slots for the 16 diagonal extractions.
    scratch = ctx.enter_context(tc.tile_pool(name="scratch", bufs=16))

    # Resident stationary operand: all local rows, transposed, [p, m, kt, c].
    a_t = singles.tile([P, M_TILES, K_TILES, P], _FP8)

    def load_a_block(m):
        nc.sync.dma_start(out=a_t[:, m], in_=lhsT[:, m])

    load_a_block(0)

    # Per-row partial sums: column m*N_CHUNKS + nch. Disjoint-column writes
    # carry no WAW dependencies between the exps.
    sums = singles.tile([P, M_TILES * N_CHUNKS], _FP32)
    # Raw (pre-exp, scaled) diagonal values: cols [0:8] positive, [8:16] self.
    diag = singles.tile([P, 2 * M_TILES], _FP32)
    # SBUF staging for the col-sums (DMA cannot read PSUM directly).
    cs_sb = singles.tile([1, len(CS_CHUNKS) * N_CHUNK], _FP32)

    # Deferred ones-matmul emissions: the col reduction of pair j is queued
    # on the PE after tile 2j+2's matmul group so the PE never waits for
    # ACT(2j+1).
    pending = []

    def flush_pending():
        while pending:
            pending.pop(0)()

    for ci, nch in enumerate(CHUNK_ORDER):
        b_t = bpool.tile([P, 2, K_TILES, N_HALF], _FP8)
        # Two half-loads so matmuls on the first PSUM bank can start while
        # the second half is still arriving.
        nc.sync.dma_start(out=b_t[:, 0], in_=brot[2 * nch])
        nc.sync.dma_start(out=b_t[:, 1], in_=brot[2 * nch + 1])
        if ci == 0:
            for mb in range(1, M_TILES):
                load_a_block(mb)
        is_cs = nch in CS_CHUNKS
        for m in range(M_TILES):
            ps = pspool.tile([P, N_CHUNK], _FP32)
            col = m * N_CHUNKS + nch
            for half in range(N_CHUNK // N_HALF):
                hs = slice(half * N_HALF, (half + 1) * N_HALF)
                for kt in range(0, K_TILES, 2):
                    nc.tensor.matmul(
                        ps[:, hs],
                        a_t[:, m, kt : kt + 2, :],
                        b_t[:, half, kt : kt + 2, :],
                        start=(kt == 0),
                        stop=(kt == K_TILES - 2),
                        perf_mode=mybir.MatmulPerfMode.DoubleRow,
                    )
            flush_pending()
            # exp over both PSUM banks at once; fused per-row accumulation.
            # All PE-group RAW waits share one semaphore.
            if m % 2 == 0:
                e2 = epool.tile([P, 2, N_CHUNK], _FP8)
            nc.scalar.activation(
                out=e2[:, m % 2],
                in_=ps,
                func=AF.Exp,
                bias=(-CS_BIAS if is_cs else -INV_T),
                scale=SIM_SCALE,
                accum_out=sums[:, col : col + 1],
            )
            if is_cs:
                if m == 0:
                    cs_t = cspool.tile([P, N_CHUNK], _FP32)
                if m % 2 == 1:

                    def emit(cs_t=cs_t, e2=e2, m=m, nch=nch):
                        # One DR ones-matmul per 512-col half (a matmul
                        # output must not cross a PSUM bank boundary),
                        # reducing m-tiles m-1 and m along partitions.
                        for half in range(N_CHUNK // N_HALF):
                            hs = slice(half * N_HALF, (half + 1) * N_HALF)
                            nc.tensor.matmul(
                                cs_t[:, hs],
                                ones,
                                e2[:, :, hs],
                                start=(m == 1),
                                stop=(m == M_TILES - 1),
                                perf_mode=mybir.MatmulPerfMode.DoubleRow,
                            )
                        if m == M_TILES - 1:
                            # Park row 0 in SBUF on the (mostly idle) DVE so
                            # the PSUM accumulator can recycle.
                            k = CS_CHUNKS.index(nch)
                            nc.vector.tensor_scalar_mul(
                                cs_sb[:, k * N_CHUNK : (k + 1) * N_CHUNK],
                                cs_t[0:1, :],
                                1.0,
                            )

                    pending.append(emit)
            # Diagonal extraction on the two special chunks. In rotated
            # coords, m-tile m's self diagonal lives at rot cols
            # [m*128, (m+1)*128) -> chunk 0; the positive-pair diagonal at
            # rot cols [4096 + m*128, ...) -> chunk 4.
            dcol = None
            if nch == 0:
                dcol = M_TILES + m
            elif nch == 4:
                dcol = m
            if dcol is not None:
                # DVE extracts the raw f32 diagonal straight from PSUM
                # (identity mul + reduce); Bacc's generate_event_semaphores
                # legalizes the resulting extra WAR wait on the recycling
                # matmul, and this keeps the ACT engine (the pipeline's
                # second-busiest) free of copy work.
                off = m * P
                diag_t = scratch.tile([P, P], _FP32)
                nc.vector.tensor_mul(diag_t, ps[:, off : off + P], ident)
                nc.vector.reduce_sum(
                    diag[:, dcol : dcol + 1], diag_t, axis=mybir.AxisListType.X
                )
        if nch == 4:
            # Both diagonals complete 2/5 into the kernel; ship them now.
            nc.sync.dma_start(out=diag_out, in_=diag)

    flush_pending()
    nc.sync.dma_start(out=sums_out, in_=sums)
    nc.sync.dma_start(out=cs_out, in_=cs_sb)
    ctx.close()


_NC_CACHE = {}


def _get_nc():
    if "nc" not in _NC_CACHE:
        _NC_CACHE["nc"] = _build_bass()
    return _NC_CACHE["nc"]


def _make_in_maps(z1, z2):
    z1 = np.asarray(z1, dtype=np.float32)
    z2 = np.asarray(z2, dtype=np.float32)
    z = np.concatenate([z1, z2], axis=0)  # [8192, 1024]
    nrm = np.sqrt(np.sum(z * z, axis=1, keepdims=True, dtype=np.float32))
    n = z / np.maximum(nrm, EPS)
    repsT = np.ascontiguousarray(n.T * FP8_SCALE).astype(_FP8_NP)  # [1024, 8192]
    in_maps = []
    for c in range(NCORES):
        p0 = c * ROWS_PER_CORE
        rolled = np.concatenate([repsT[:, p0:], repsT[:, :p0]], axis=1)
        cols = rolled[:, :N_COLS]  # rot cols [0, 5120)
        lhsT_c = repsT[:, c * ROWS_PER_CORE : (c + 1) * ROWS_PER_CORE]
        # [p, m, kt, col]
        lhsT_blk = np.ascontiguousarray(
            lhsT_c.reshape(K_TILES, P, M_TILES, P).transpose(1, 2, 0, 3)
        )
        # Block per 512-col half: [half, p, kt, col]
        b_blk = np.ascontiguousarray(
            cols.reshape(K_TILES, P, N_COLS // N_HALF, N_HALF).transpose(2, 1, 0, 3)
        )
        in_maps.append({"lhst": lhsT_blk, "brot": b_blk})
    return in_maps


def _combine(results):
    # Per row i: S_i = local row sums + col-sum contributions from the three
    # cores whose d=1..3 blocks hit column i. Then
    #   T_i = S_i - e_self_i + e_pos_i;  loss_row = ln(T_i) - (pos_i - 10)
    # with pos_i - 10 = SIM_SCALE*draw - 10. A few K flops; done in f64.
    # Chunks 1..3 were computed as exp(s*x - CS_BIAS): rescale by
    # e^(CS_BIAS - INV_T).
    resc = np.exp(CS_BIAS - INV_T)
    S_loc = np.empty(S, dtype=np.float64)
    CS = np.zeros(S, dtype=np.float64)
    draw = np.empty(S, dtype=np.float64)
    dself = np.empty(S, dtype=np.float64)
    ncs = len(CS_CHUNKS) * N_CHUNK
    for c, r in enumerate(results):
        st = r["sums"].astype(np.float64).reshape(P, M_TILES, N_CHUNKS)
        stot = st[:, :, [0, 4]].sum(axis=2) + resc * st[:, :, 1:4].sum(axis=2)
        sl = slice(c * ROWS_PER_CORE, (c + 1) * ROWS_PER_CORE)
        S_loc[sl] = stot.T.reshape(-1)  # local row l = m*128 + p
        diag = r["diag"].astype(np.float64)
        draw[sl] = diag[:, :M_TILES].T.reshape(-1)
        dself[sl] = diag[:, M_TILES:].T.reshape(-1)
        cs = r["colsums"].astype(np.float64).reshape(-1)  # rot cols [1024, 4096)
        idx = (np.arange(ncs) + c * ROWS_PER_CORE + N_CHUNK) % S
        CS[idx] += resc * cs
    Stot = S_loc + CS
    e_pos = np.exp(SIM_SCALE * draw - INV_T)
    e_self = np.exp(SIM_SCALE * dself - INV_T)
    loss_rows = np.log(Stot - e_self + e_pos) - (SIM_SCALE * draw - INV_T)
    return np.array(loss_rows.mean(), dtype=np.float32)


def run_traced(z1, z2, **spmd_kwargs):
    """Run on HW with profiling; returns (loss, BassKernelResults)."""
    nc = _get_nc()
    in_maps = _make_in_maps(z1, z2)
    res = bass_utils.run_bass_kernel_spmd(
        nc, in_maps, core_ids=list(range(NCORES)), trace=True, **spmd_kwargs
    )
    return _combine(res.results), res


def kernel(z1, z2):
    nc = _get_nc()
    in_maps = _make_in_maps(z1, z2)
    last_err = None
    for _attempt in range(3):
        try:
            res = bass_utils.run_bass_kernel_spmd(
                nc, in_maps, core_ids=list(range(NCORES))
            )
            return _combine(res.results)
        except Exception as e:  # transient device wedge: retry
            last_err = e
            time.sleep(2.0)
    raise last_err


# revision 11
# speedup vs baseline: 1.0865x; 1.0283x over previous
"""Contrastive loss (SimCLR-style NT-Xent) Trainium2 kernel.

Full inputs z1, z2: [4096, 1024] f32. Output: scalar f32 loss.

Strategy (8 NeuronCores, SPMD, no collectives), exploiting sim symmetry:
  - Host: L2-normalize rows of reps = concat(z1, z2)  [8192, 1024] (f32),
    transpose to repsT [1024, 8192], scale by 256 and cast to fp8e4m3.
  - sim = reps @ reps.T is symmetric, so each unordered pair only needs to
    be computed once. Circulant block scheme: core c computes rows
    R_c = [c*1024, (c+1)*1024) against column blocks d = 0..4, i.e. actual
    columns [c*1024, c*1024 + 5120) mod 8192 — and within d=0 only the
    upper triangle (cols >= m*128 for row tile m).
      * d=0 (cols R_c): upper triangle only. Row-sums cover pairs with
        j >= tile(i); the strictly-lower mirror terms come from per-column
        sums of exp over the tiles above the diagonal (exported to host).
      * d=1..3: the mirror blocks [R_{c+d}, R_c] are computed by nobody, so
        core c exports per-column sums of exp over these blocks; the host
        adds them into S_j for the rows owned by cores c+1..c+3.
      * d=4: cores c and c+4 both compute their mutual block, so row-sums
        alone cover both sides. The positive-pair diagonal lives here
        (rot col 4096 + r for local row r); the self diagonal in d=0.
  - Per core the program is identical (SPMD); feeding core c its columns
    rotated by p0 = c*1024 puts the self diagonal at rot cols [0, 1024)
    and the positive diagonal at rot cols [4096, 5120) for every core.
  - Device, per (m-tile of 128 rows, n-chunk of 1024 cols): fp8 DoubleRow
    matmuls accumulate K=1024 in 4 instructions per 512-col PSUM bank; ACT
    exp with fused per-row accumulation (accum_out). Chunks 0/4 use
    exp(s*x - 10); chunks 1..3 use exp(s*x - 1) written as fp8e4m3 (values
    ~e^{10 cos} stay in fp8 range; the host rescales by e^-9) into per-PAIR
    tiles [128, 2, 1024] so a single fp8 DoubleRow ones-matmul per pair
    reduces TWO m-tiles along partitions at once into the per-chunk PSUM
    col-sum accumulator. Chunk 0's mirror col-sums use bf16 ones-matmuls
    with per-m variable column ranges (tiles strictly above the diagonal).
    Each ones-matmul is emitted after the NEXT tile's matmul group so the
    in-order PE queue never stalls waiting for ACT.
  - Chunk processing order [1, 2, 3, 0, 4]: the col-sum chunks finish and
    ship mid-kernel; the tail is just the last exp + diag/sums DMAs.
  - DMA layouts are fully contiguous per partition (1KB a-blocks, 4KB
    b-halves); the first processed half is split into four 2-ktile strips
    so the PE starts ~3us earlier.
  - Host: S_i = (local row sums) + (col-sum contributions);
    T = S - e_self + e_pos; loss = mean(ln(T) - (pos - 10)).
"""

import time
from contextlib import ExitStack

import numpy as np
import ml_dtypes

import concourse.bass as bass
import concourse.tile as tile
from concourse import bacc
from concourse import mybir
from concourse import bass_utils
from concourse.masks import make_identity

B = 4096
D = 1024
S = 2 * B  # 8192 rows/cols of sim
NCORES = 8
ROWS_PER_CORE = S // NCORES  # 1024
P = 128
M_TILES = ROWS_PER_CORE // P  # 8
K_TILES = D // P  # 8
N_CHUNK = 1024  # two PSUM banks per (m, chunk) tile
N_CHUNKS = 5  # symmetric scheme: blocks d=0..4 only
N_COLS = N_CHUNKS * N_CHUNK  # 5120 columns per core
N_HALF = 512  # max matmul moving free dim into one PSUM bank
CS_CHUNKS = (1, 2, 3)  # chunks whose col-sums are exported (fp8 path)
CS_SLOTS = 4  # 3 cs chunks + chunk-0 triangle mirror sums
CHUNK_ORDER = (1, 2, 3, 0, 4)
INV_T = 10.0  # 1 / temperature
CS_BIAS = 1.0  # cs chunks compute exp(s*x - CS_BIAS) to fit fp8e4m3
EPS = 1e-12
FP8_SCALE = 256.0  # input scale: keeps fp8e4m3 operands in their sweet spot
SIM_SCALE = INV_T / (FP8_SCALE * FP8_SCALE)  # exp(SIM_SCALE * raw - bias)

_FP32 = mybir.dt.float32
_FP8 = mybir.dt.float8e4
_BF16 = mybir.dt.bfloat16
_FP8_NP = mybir.dt.np(_FP8)


def _build_bass():
    # Bacc (not raw Bass): its compile() runs generate_event_semaphores,
    # which splits multi-semaphore waits into standalone EventSemaphore
    # instructions — engine instructions can encode only one wait.
    nc = bacc.Bacc("TRN2", debug=False, num_devices=NCORES, enable_partition_id=False)
    # lhsT stored [p, m, kt, col]: both the DMA src and dst are contiguous
    # 1KB runs per partition per m-block.
    lhsT = nc.dram_tensor(
        "lhst", [P, M_TILES, K_TILES, P], _FP8, kind="ExternalInput"
    ).ap()
    # brot blocked per 512-column half: [half, p, kt, col]; contiguous 4KB
    # per partition per half.
    brot = nc.dram_tensor(
        "brot", [N_COLS // N_HALF, P, K_TILES, N_HALF], _FP8, kind="ExternalInput"
    ).ap()
    # Raw reductions out; the tiny final combine (a few K flops) runs on the
    # host, which avoids a 1.3us ACT table switch (Ln) in the device tail.
    sums_out = nc.dram_tensor(
        "sums", [P, M_TILES * N_CHUNKS], _FP32, kind="ExternalOutput"
    ).ap()
    diag_out = nc.dram_tensor(
        "diag", [P, 2 * M_TILES], _FP32, kind="ExternalOutput"
    ).ap()
    # Per-column exp sums: slots 0..2 = chunks 1..3 (e^{-CS_BIAS} units),
    # slot 3 = chunk-0 triangle mirrors (e^{-10} units, cols 0..127 unused).
    cs_out = nc.dram_tensor(
        "colsums", [1, CS_SLOTS * N_CHUNK], _FP32, kind="ExternalOutput"
    ).ap()

    # Pre-TileContext const region (same pattern as Bass.__init__'s
    # const_aps): values read by hot-loop instructions with no tracked
    # dependency, so they add no per-instruction sync waits. Instead of a
    # full all-engine barrier (~3us butterfly), hand off with one semaphore
    # to the only consumers (ACT reads the bias consts, DVE the identity,
    # PE the ones blocks).
    bias_th = nc.alloc_sbuf_tensor("const-f32-neg10", [P, 1], _FP32)
    nc.gpsimd.memset(bias_th.ap(), -INV_T)
    nc.const_aps.aps[(_FP32, -INV_T)] = bias_th.ap()
    bias2_th = nc.alloc_sbuf_tensor("const-f32-csbias", [P, 1], _FP32)
    nc.gpsimd.memset(bias2_th.ap(), -CS_BIAS)
    nc.const_aps.aps[(_FP32, -CS_BIAS)] = bias2_th.ap()
    ident_th = nc.alloc_sbuf_tensor("identity-f32", [P, P], _FP32)
    nc.gpsimd.memset(ident_th.ap(), 0.0)
    ones8_th = nc.alloc_sbuf_tensor("ones-fp8", [P, 2, P], _FP8)
    nc.gpsimd.memset(ones8_th.ap(), 1.0)
    ones16_th = nc.alloc_sbuf_tensor("ones-bf16", [P, P], _BF16)
    nc.gpsimd.memset(ones16_th.ap(), 1.0)
    ident_inst = nc.gpsimd.affine_select(
        out=ident_th.ap(),
        in_=ident_th.ap(),
        compare_op=mybir.AluOpType.not_equal,
        fill=1.0,
        base=0,
        pattern=[[-1, P]],
        channel_multiplier=1,
    )
    const_sem = nc.alloc_semaphore("const-ready")
    ident_inst.then_inc(const_sem, 1)
    nc.vector.wait_ge(const_sem, 1)
    nc.scalar.wait_ge(const_sem, 1)
    nc.tensor.wait_ge(const_sem, 1)

    with tile.TileContext(nc) as tc:
        _body(
            tc,
            lhsT,
            brot,
            sums_out,
            diag_out,
            cs_out,
            ident_th.ap(),
            ones8_th.ap(),
            ones16_th.ap(),
        )
    nc.compile()
    return nc


def _body(tc, lhsT, brot, sums_out, diag_out, cs_out, ident, ones8, ones16):
    nc = tc.nc
    AF = mybir.ActivationFunctionType

    ctx = ExitStack()
    singles = ctx.enter_context(tc.tile_pool(name="singles", bufs=1))
    bpool = ctx.enter_context(tc.tile_pool(name="bchunks", bufs=3))
    # 3 tiles x 2 banks for the matmul pipeline + 2 banks for the col-sum
    # accumulator = all 8 PSUM banks.
    pspool = ctx.enter_context(tc.tile_pool(name="psum", bufs=3, space="PSUM"))
    cspool = ctx.enter_context(tc.tile_pool(name="psum-cs", bufs=1, space="PSUM"))
    # Exp outputs per m-tile PAIR: [p, 2, 1024] fp8 so one DoubleRow
    # ones-matmul reduces both tiles along partitions at once (chunks 1-3).
    epool = ctx.enter_context(tc.tile_pool(name="exps", bufs=4))
    # bf16 exp tiles for chunk 0's variable-width triangle reduction.
    e0pool = ctx.enter_context(tc.tile_pool(name="exps0", bufs=3))
    # Single-use slots for the 16 diagonal extractions.
    scratch = ctx.enter_context(tc.tile_pool(name="scratch", bufs=16))

    # Resident stationary operand: all local rows, transposed, [p, m, kt, c].
    a_t = singles.tile([P, M_TILES, K_TILES, P], _FP8)

    def load_a_block(m):
        nc.sync.dma_start(out=a_t[:, m], in_=lhsT[:, m])

    load_a_block(0)

    # Per-row partial sums: column m*N_CHUNKS + nch. Disjoint-column writes
    # carry no WAW dependencies between the exps.
    sums = singles.tile([P, M_TILES * N_CHUNKS], _FP32)
    # Raw (pre-exp, scaled) diagonal values: cols [0:8] positive, [8:16] self.
    diag = singles.tile([P, 2 * M_TILES], _FP32)
    # SBUF staging for the col-sums (DMA cannot read PSUM directly).
    cs_sb = singles.tile([1, CS_SLOTS * N_CHUNK], _FP32)

    # Deferred ones-matmul emissions: the col reduction of a tile (pair) is
    # queued on the PE after the NEXT tile's matmul group so the PE never
    # waits on ACT.
    pending = []

    def flush_pending():
        while pending:
            pending.pop(0)()

    for ci, nch in enumerate(CHUNK_ORDER):
        b_t = bpool.tile([P, 2, K_TILES, N_HALF], _FP8)
        if ci == 0:
            # Split the very first half into 2-ktile strips: the first
            # matmul group can start after 128KB instead of 512KB.
            for kt in range(0, K_TILES, 2):
                nc.sync.dma_start(
                    out=b_t[:, 0, kt : kt + 2], in_=brot[2 * nch][:, kt : kt + 2]
                )
        else:
            nc.sync.dma_start(out=b_t[:, 0], in_=brot[2 * nch])
        nc.sync.dma_start(out=b_t[:, 1], in_=brot[2 * nch + 1])
        if ci == 0:
            for mb in range(1, M_TILES):
                load_a_block(mb)
        is_cs = nch in CS_CHUNKS
        for m in range(M_TILES):
            ps = pspool.tile([P, N_CHUNK], _FP32)
            col = m * N_CHUNKS + nch
            # Chunk 0: only the upper triangle (cols >= m*128).
            c_lo = m * P if nch == 0 else 0
            for half in range(N_CHUNK // N_HALF):
                h0, h1 = half * N_HALF, (half + 1) * N_HALF
                lo = max(c_lo, h0)
                if lo >= h1:
                    continue
                for kt in range(0, K_TILES, 2):
                    nc.tensor.matmul(
                        ps[:, lo:h1],
                        a_t[:, m, kt : kt + 2, :],
                        b_t[:, half, kt : kt + 2, lo - h0 : N_HALF],
                        start=(kt == 0),
                        stop=(kt == K_TILES - 2),
                        perf_mode=mybir.MatmulPerfMode.DoubleRow,
                    )
            flush_pending()
            # exp; fused per-row accumulation. All PE-group RAW waits share
            # one semaphore.
            if nch == 0:
                e0_t = e0pool.tile([P, N_CHUNK], _BF16)
                e_ap = e0_t[:, c_lo:]
            else:
                if m % 2 == 0:
                    e2 = epool.tile([P, 2, N_CHUNK], _FP8)
                e_ap = e2[:, m % 2]
            nc.scalar.activation(
                out=e_ap,
                in_=ps[:, c_lo:],
                func=AF.Exp,
                bias=(-CS_BIAS if is_cs else -INV_T),
                scale=SIM_SCALE,
                accum_out=sums[:, col : col + 1],
            )
            if is_cs:
                if m == 0:
                    cs_t = cspool.tile([P, N_CHUNK], _FP32)
                if m % 2 == 1:

                    def emit(cs_t=cs_t, e2=e2, m=m, nch=nch):
                        # One DR ones-matmul per 512-col half (a matmul
                        # output must not cross a PSUM bank boundary),
                        # reducing m-tiles m-1 and m along partitions.
                        for half in range(N_CHUNK // N_HALF):
                            hs = slice(half * N_HALF, (half + 1) * N_HALF)
                            nc.tensor.matmul(
                                cs_t[:, hs],
                                ones8,
                                e2[:, :, hs],
                                start=(m == 1),
                                stop=(m == M_TILES - 1),
                                perf_mode=mybir.MatmulPerfMode.DoubleRow,
                            )
                        if m == M_TILES - 1:
                            # Park row 0 in SBUF on the (mostly idle) DVE so
                            # the PSUM accumulator can recycle.
                            k = CS_CHUNKS.index(nch)
                            nc.vector.tensor_scalar_mul(
                                cs_sb[:, k * N_CHUNK : (k + 1) * N_CHUNK],
                                cs_t[0:1, :],
                                1.0,
                            )

                    pending.append(emit)
            elif nch == 0 and m < M_TILES - 1:
                # Triangle mirror col-sums: reduce e0(m) over the tiles
                # strictly above the diagonal (global cols [(m+1)*128,1024)),
                # accumulated per PSUM bank with per-bank start/stop.
                if m == 0:
                    # Same variable name as the cs-chunk accumulator on
                    # purpose: tile tags are inferred from the assignee, and
                    # sharing the tag lets both rotate through ONE PSUM slot.
                    cs_t = cspool.tile([P, N_CHUNK], _FP32)

                def emit0(cs0_t=cs_t, e_ap=e_ap, m=m, c_lo=c_lo):
                    g0 = (m + 1) * P
                    for h0 in (0, N_HALF):
                        h1 = h0 + N_HALF
                        lo = max(g0, h0)
                        if lo >= h1:
                            continue
                        # Last contributor to bank [h0,h1): the largest m
                        # with (m+1)*128 < h1.
                        last_m = (h1 - P) // P - 1
                        nc.tensor.matmul(
                            cs0_t[:, lo:h1],
                            ones16,
                            e_ap[:, lo - c_lo : h1 - c_lo],
                            start=(m == 0),
                            stop=(m == last_m),
                        )
                    if m == M_TILES - 2:
                        nc.vector.tensor_scalar_mul(
                            cs_sb[:, 3 * N_CHUNK : 4 * N_CHUNK],
                            cs0_t[0:1, :],
                            1.0,
                        )
                        nc.sync.dma_start(out=cs_out, in_=cs_sb)

                pending.append(emit0)
            # Diagonal extraction on the two special chunks. In rotated
            # coords, m-tile m's self diagonal lives at rot cols
            # [m*128, (m+1)*128) -> chunk 0; the positive-pair diagonal at
            # rot cols [4096 + m*128, ...) -> chunk 4.
            dcol = None
            if nch == 0:
                dcol = M_TILES + m
            elif nch == 4:
                dcol = m
            if dcol is not None:
                # DVE extracts the raw f32 diagonal straight from PSUM
                # (identity mul + reduce); Bacc's generate_event_semaphores
                # legalizes the resulting extra WAR wait on the recycling
                # matmul, and this keeps the ACT engine (the pipeline's
                # second-busiest) free of copy work.
                off = m * P
                diag_t = scratch.tile([P, P], _FP32)
                nc.vector.tensor_mul(diag_t, ps[:, off : off + P], ident)
                nc.vector.reduce_sum(
                    diag[:, dcol : dcol + 1], diag_t, axis=mybir.AxisListType.X
                )

    flush_pending()
    nc.sync.dma_start(out=diag_out, in_=diag)
    nc.sync.dma_start(out=sums_out, in_=sums)
    ctx.close()


_NC_CACHE = {}


def _get_nc():
    if "nc" not in _NC_CACHE:
        _NC_CACHE["nc"] = _build_bass()
    return _NC_CACHE["nc"]


def _make_in_maps(z1, z2):
    z1 = np.asarray(z1, dtype=np.float32)
    z2 = np.asarray(z2, dtype=np.float32)
    z = np.concatenate([z1, z2], axis=0)  # [8192, 1024]
    nrm = np.sqrt(np.sum(z * z, axis=1, keepdims=True, dtype=np.float32))
    n = z / np.maximum(nrm, EPS)
    repsT = np.ascontiguousarray(n.T * FP8_SCALE).astype(_FP8_NP)  # [1024, 8192]
    in_maps = []
    for c in range(NCORES):
        p0 = c * ROWS_PER_CORE
        rolled = np.concatenate([repsT[:, p0:], repsT[:, :p0]], axis=1)
        cols = rolled[:, :N_COLS]  # rot cols [0, 5120)
        lhsT_c = repsT[:, c * ROWS_PER_CORE : (c + 1) * ROWS_PER_CORE]
        # [p, m, kt, col]
        lhsT_blk = np.ascontiguousarray(
            lhsT_c.reshape(K_TILES, P, M_TILES, P).transpose(1, 2, 0, 3)
        )
        # Block per 512-col half: [half, p, kt, col]
        b_blk = np.ascontiguousarray(
            cols.reshape(K_TILES, P, N_COLS // N_HALF, N_HALF).transpose(2, 1, 0, 3)
        )
        in_maps.append({"lhst": lhsT_blk, "brot": b_blk})
    return in_maps


def _combine(results):
    # Per row i: S_i = local row sums + col-sum contributions (mirror terms
    # from the d=1..3 blocks of three other cores, plus the local d=0
    # triangle mirrors). Then
    #   T_i = S_i - e_self_i + e_pos_i;  loss_row = ln(T_i) - (pos_i - 10)
    # with pos_i - 10 = SIM_SCALE*draw - 10. A few K flops; done in f64.
    # Chunks 1..3 were computed as exp(s*x - CS_BIAS): rescale by
    # e^(CS_BIAS - INV_T).
    resc = np.exp(CS_BIAS - INV_T)
    S_loc = np.empty(S, dtype=np.float64)
    CS = np.zeros(S, dtype=np.float64)
    draw = np.empty(S, dtype=np.float64)
    dself = np.empty(S, dtype=np.float64)
    ncs = len(CS_CHUNKS) * N_CHUNK
    for c, r in enumerate(results):
        st = r["sums"].astype(np.float64).reshape(P, M_TILES, N_CHUNKS)
        stot = st[:, :, [0, 4]].sum(axis=2) + resc * st[:, :, 1:4].sum(axis=2)
        sl = slice(c * ROWS_PER_CORE, (c + 1) * ROWS_PER_CORE)
        S_loc[sl] = stot.T.reshape(-1)  # local row l = m*128 + p
        diag = r["diag"].astype(np.float64)
        draw[sl] = diag[:, :M_TILES].T.reshape(-1)
        dself[sl] = diag[:, M_TILES:].T.reshape(-1)
        cs = r["colsums"].astype(np.float64).reshape(-1)
        idx = (np.arange(ncs) + c * ROWS_PER_CORE + N_CHUNK) % S
        CS[idx] += resc * cs[:ncs]
        # chunk-0 triangle mirrors (e^{-10} units), cols 128..1023 of R_c
        CS[c * ROWS_PER_CORE + P : (c + 1) * ROWS_PER_CORE] += cs[
            3 * N_CHUNK + P : 4 * N_CHUNK
        ]
    Stot = S_loc + CS
    e_pos = np.exp(SIM_SCALE * draw - INV_T)
    e_self = np.exp(SIM_SCALE * dself - INV_T)
    loss_rows = np.log(Stot - e_self + e_pos) - (SIM_SCALE * draw - INV_T)
    return np.array(loss_rows.mean(), dtype=np.float32)


def run_traced(z1, z2, **spmd_kwargs):
    """Run on HW with profiling; returns (loss, BassKernelResults)."""
    nc = _get_nc()
    in_maps = _make_in_maps(z1, z2)
    res = bass_utils.run_bass_kernel_spmd(
        nc, in_maps, core_ids=list(range(NCORES)), trace=True, **spmd_kwargs
    )
    return _combine(res.results), res


def kernel(z1, z2):
    nc = _get_nc()
    in_maps = _make_in_maps(z1, z2)
    last_err = None
    for _attempt in range(3):
        try:
            res = bass_utils.run_bass_kernel_spmd(
                nc, in_maps, core_ids=list(range(NCORES))
            )
            return _combine(res.results)
        except Exception as e:  # transient device wedge: retry
            last_err = e
            time.sleep(2.0)
    raise last_err
